# revision 62
# baseline (speedup 1.0000x reference)
"""Causal self-attention (B=4, T=2048, C=1024, H=16) on 8 TRN2 NeuronCores.

Sharding: tensor-parallel over heads. Core c owns heads (2c, 2c+1) for all
batches: QKV projections are column-sharded, attention is embarrassingly
parallel over (batch, head), out_proj is contraction-sharded and the host
sums the 8 partial outputs (the unshard step for a contraction shard).

Single-core schedule: a software pipeline over batches.  For each batch b:
projections+RoPE for b's 2048 tokens, then attention (both heads), with the
NEXT batch's projection work and the PREVIOUS batch's out_proj tiles
interleaved into the attention kb-loop as "filler" so the PE never idles
while the ACT engine runs the softmax exp stream.

Key device-level tricks:
  - S^T tiles for the two heads are computed by a row-tiled PAIR of matmuls
    (tile_position (0,0) and (64,0)): each head contracts over its own 64
    partitions, so both matmuls run concurrently on the PE array, writing
    the two halves of one [128, 1024] PSUM tile.
  - exp runs on ACT over the joint [128, 1024] tile (one instruction for
    both heads) with scale=1/8; max-subtraction is skipped (|S|/8 <= ~3.2
    for this operator).  The causal staircase is a 0/1 bf16 mask multiply
    on DVE, column-trimmed to the triangle's actual width.
  - V is transposed token-major 128x128 (both heads at once) on the PE; the
    vtm layout [128, gkb, 2, 66] keeps each head's 64 dims + a ones column
    contiguous, so the PV matmul (lhsT = vtm[:, gkb, j, 0:65]) emits both
    O^T rows and the softmax denominator in one accumulation.
  - out_proj partials are written bf16 (halves the output DMA); the host
    sums the 8 partials in fp32, adds bo, and transposes back.
  - ACT does exp only (plus half the out_proj PSUM drains); projections'
    RoPE outputs, V bias, O drains and normalization run on DVE/Pool.
"""

import numpy as np
import ml_dtypes

import concourse.bass as bass
import concourse.mybir as mybir
import concourse.tile as tile
from concourse import bacc
from concourse.bass_utils import run_bass_kernel_spmd
from concourse.masks import make_identity

BF16 = mybir.dt.bfloat16
F32 = mybir.dt.float32
AT = mybir.ActivationFunctionType
OP = mybir.AluOpType

B, T, C, H = 4, 2048, 1024, 16
DH = 64
BT = B * T            # 8192
NCORES = 8
NKB = T // 128        # 16 k-blocks per batch
NQT = T // 512        # 4 q-tiles per batch

_NC = None            # cached compiled Bass module


def _build_nc(repeat=1, phases="BCD"):
    nc = bacc.Bacc("TRN2", target_bir_lowering=False, debug=False)

    xT = nc.declare_dram_parameter("xT", [C, BT], BF16, isOutput=False)
    wq = nc.declare_dram_parameter("wq", [C, 128], BF16, isOutput=False)
    wk = nc.declare_dram_parameter("wk", [C, 128], BF16, isOutput=False)
    wv = nc.declare_dram_parameter("wv", [C, 128], BF16, isOutput=False)
    wo = nc.declare_dram_parameter("wo", [128, C], BF16, isOutput=False)
    bq = nc.declare_dram_parameter("bq", [128, 1], F32, isOutput=False)
    bk = nc.declare_dram_parameter("bk", [128, 1], F32, isOutput=False)
    bv = nc.declare_dram_parameter("bv", [128, 1], F32, isOutput=False)
    csa = nc.declare_dram_parameter("csa", [128, T], BF16, isOutput=False)
    csb = nc.declare_dram_parameter("csb", [128, T], BF16, isOutput=False)
    btri = nc.declare_dram_parameter("btri", [128, 128], BF16, isOutput=False)
    ca = nc.declare_dram_parameter("ca", [128, 128], BF16, isOutput=False)
    cb = nc.declare_dram_parameter("cb", [128, 128], BF16, isOutput=False)
    outT = nc.declare_dram_parameter("outT", [C, BT], BF16, isOutput=True)

    from contextlib import ExitStack
    from collections import deque
    with tile.TileContext(nc) as tc, ExitStack() as ctx:
        const = ctx.enter_context(tc.tile_pool(name="const", bufs=1))
        xpool = ctx.enter_context(tc.tile_pool(name="xpool", bufs=4))
        ptp = ctx.enter_context(tc.tile_pool(name="ptp", bufs=8))
        rtmp = ctx.enter_context(tc.tile_pool(name="rtmp", bufs=4))
        small = ctx.enter_context(tc.tile_pool(name="small", bufs=3))
        otp = ctx.enter_context(tc.tile_pool(name="otp", bufs=4))
        psA = ctx.enter_context(tc.tile_pool(name="psA", bufs=2, space="PSUM"))
        psS = ctx.enter_context(tc.tile_pool(name="psS", bufs=2, space="PSUM"))
        psO = ctx.enter_context(tc.tile_pool(name="psO", bufs=2, space="PSUM"))

        # ---- constants ----
        # Only what batch 0 / token-tile 0's first matmuls need is loaded up
        # front; everything else is staged between the first token-tile's
        # filler items (the HWDGE serves DMAs roughly in issue order, so a
        # big table in front of the first x tile delays the whole pipeline).
        wq_sb = const.tile([128, 8, 128], BF16, tag="wq")
        nc.sync.dma_start(out=wq_sb, in_=wq.rearrange("(kb p) m -> p kb m", p=128))
        wk_sb = const.tile([128, 8, 128], BF16, tag="wk")
        wv_sb = const.tile([128, 8, 128], BF16, tag="wv")
        wo_sb = const.tile([128, 8, 128], BF16, tag="wo")
        csa_sb = const.tile([128, T], BF16, tag="csa")
        csb_sb = const.tile([128, T], BF16, tag="csb")
        btri_sb = const.tile([128, 128], BF16, tag="btri")
        ca_sb = const.tile([128, 128], BF16, tag="ca")
        cb_sb = const.tile([128, 128], BF16, tag="cb")
        bq_sb = const.tile([128, 1], F32, tag="bq")
        nc.sync.dma_start(out=bq_sb, in_=bq[:, :])
        bk_sb = const.tile([128, 1], F32, tag="bk")
        bv_sb = const.tile([128, 1], F32, tag="bv")
        nc.sync.dma_start(out=btri_sb, in_=btri[:, :])

        staged_dmas = {
            0: lambda: (
                nc.sync.dma_start(out=csa_sb[:, 0:512], in_=csa[:, 0:512]),
                nc.sync.dma_start(out=csb_sb[:, 0:512], in_=csb[:, 0:512]),
                nc.sync.dma_start(out=ca_sb, in_=ca[:, :]),
                nc.sync.dma_start(out=cb_sb, in_=cb[:, :]),
            ),
            1: lambda: (
                nc.sync.dma_start(
                    out=wk_sb,
                    in_=wk.rearrange("(kb p) m -> p kb m", p=128)),
                nc.sync.dma_start(out=bk_sb, in_=bk[:, :]),
            ),
            3: lambda: (
                nc.sync.dma_start(
                    out=wv_sb,
                    in_=wv.rearrange("(kb p) m -> p kb m", p=128)),
                nc.sync.dma_start(out=bv_sb, in_=bv[:, :]),
            ),
            5: lambda: (
                nc.sync.dma_start(out=csa_sb[:, 512:T], in_=csa[:, 512:T]),
                nc.sync.dma_start(out=csb_sb[:, 512:T], in_=csb[:, 512:T]),
            ),
            6: lambda: (
                nc.sync.dma_start(
                    out=wo_sb,
                    in_=wo.rearrange("p (ob m) -> p ob m", m=128)),
            ),
        }

        ident = const.tile([128, 128], BF16, tag="id")
        make_identity(nc, ident)

        QT = const.tile([128, BT], BF16, tag="QT")
        KT = const.tile([128, BT], BF16, tag="KT")
        VT = const.tile([128, BT], BF16, tag="VT")
        yT = const.tile([128, BT], BF16, tag="yT")
        # token-major V: [k-part, gkb, head, 64 dims + 64 ones]. The ones
        # block makes the PV matmul emit the softmax denominator replicated
        # on partitions 64:128, so normalize needs no partition_broadcast.
        vtm = const.tile([128, B * NKB, 2, 128], BF16, tag="vtm")
        nc.vector.memset(vtm[:, :, :, 64:128], 1.0)

        def emit_body(rep=0):
            # ---------- phase B: projections + RoPE + V transpose ----------
            def b_items(b):
                """Generator of filler closures for batch b's projections.

                All tile() allocations happen inside the closures (at pop
                time) so pool-slot WAR deps follow emission order; tiles
                shared between closures of one token-tile pass via `cell`.
                """
                for tt in range(NQT * b, NQT * b + NQT):
                    ts_ = slice(tt * 512, tt * 512 + 512)
                    pos = slice((tt % 4) * 512, (tt % 4) * 512 + 512)
                    cell = {}

                    def dma_x(cell=cell, ts_=ts_, tt=tt):
                        xt = xpool.tile([128, 8, 512], BF16, tag="xt",
                                        name=f"xt_{tt}")
                        cell["xt"] = xt
                        # Alternate the two HWDGE trigger queues so x loads
                        # overlap each other and never all serialize behind
                        # the outT stores or the big constant DMAs. Two-half
                        # split: mm4(0) only needs k-blocks 0:4.
                        eng = nc.scalar if tt % 2 == 0 else nc.sync
                        src = xT.rearrange("(kb p) m -> p kb m", p=128)
                        eng.dma_start(out=xt[:, 0:4, :], in_=src[:, 0:4, ts_])
                        eng.dma_start(out=xt[:, 4:8, :], in_=src[:, 4:8, ts_])
                    yield dma_x

                    def make_proj(w_sb, b_sb, dstT, cell=cell, tt=tt, pos=pos,
                                  ts_=ts_):
                        pk = f"pp_{dstT.name}"

                        def mm4(lo):
                            if lo == 0:
                                cell[pk] = psA.tile([128, 512], F32, tag="proj",
                                                    name=f"{pk}_{tt}")
                            pp = cell[pk]
                            xt = cell["xt"]
                            for kb in range(lo, lo + 4):
                                nc.tensor.matmul(pp, w_sb[:, kb, :], xt[:, kb, :],
                                                 start=(kb == 0), stop=(kb == 7))

                        def r_stst():
                            # DVE half of RoPE, issued one filler slot before
                            # the PE combine so the stst latency is hidden
                            pp = cell[pk]
                            ta = rtmp.tile([128, 512], BF16, tag="ta",
                                           name=f"ta_{tt}_{dstT.name}")
                            tb = rtmp.tile([128, 512], BF16, tag="tb",
                                           name=f"tb_{tt}_{dstT.name}")
                            cell[pk + "ab"] = (ta, tb)
                            nc.vector.scalar_tensor_tensor(
                                out=ta, in0=pp, scalar=b_sb[:, 0:1],
                                in1=csa_sb[:, pos], op0=OP.add, op1=OP.mult)
                            nc.vector.scalar_tensor_tensor(
                                out=tb, in0=pp, scalar=b_sb[:, 0:1],
                                in1=csb_sb[:, pos], op0=OP.add, op1=OP.mult)

                        def mm4b_stst():
                            mm4(4)
                            r_stst()

                        def r_epi():
                            ta, tb = cell[pk + "ab"]
                            rp = psA.tile([128, 512], F32, tag="proj",
                                          name=f"rp_{tt}_{dstT.name}")
                            nc.tensor.matmul(rp, ca_sb, ta, start=True,
                                             stop=False)
                            nc.tensor.matmul(rp, cb_sb, tb, start=False,
                                             stop=True)
                            nc.vector.tensor_copy(dstT[:, ts_], rp)

                        def v_epi():
                            nc.vector.tensor_scalar_add(VT[:, ts_], cell[pk],
                                                        b_sb[:, 0:1])

                        return mm4, mm4b_stst, r_epi, v_epi

                    q_mm4, q_mm4b, q_epi, _ = make_proj(wq_sb, bq_sb, QT)
                    k_mm4, k_mm4b, k_epi, _ = make_proj(wk_sb, bk_sb, KT)
                    v_mm4, _, _, v_epi = make_proj(wv_sb, bv_sb, VT)

                    def v_tr_fn(half, tt=tt):
                        def v_tr():
                            tr = psA.tile([128, 512], F32, tag="proj",
                                          name=f"tr_{tt}_{half}")
                            trb = tr.bitcast(BF16)  # [128, 1024] bf16 view
                            for s in range(2):
                                sub = half * 2 + s
                                gkb = tt * 4 + sub
                                col = slice(tt * 512 + sub * 128,
                                            tt * 512 + sub * 128 + 128)
                                nc.tensor.transpose(
                                    trb[:, 128 * s:128 * s + 128],
                                    VT[:, col], ident)
                                nc.vector.tensor_copy(
                                    vtm[:, gkb, :, 0:64],
                                    trb[:, 128 * s:128 * s + 128]
                                    .rearrange("p (j d) -> p j d", j=2))
                        return v_tr

                    # order interleaves the three projections so every DVE
                    # producer (stst, v_epi) has a PE item between it and its
                    # PE consumer (rp combine / transpose)
                    yield (lambda f=q_mm4: f(0))
                    yield q_mm4b
                    yield (lambda f=k_mm4: f(0))
                    yield q_epi
                    yield k_mm4b
                    yield (lambda f=v_mm4: f(0))
                    yield k_epi
                    yield (lambda f=v_mm4: f(4))
                    yield v_epi
                    yield v_tr_fn(0)
                    yield v_tr_fn(1)

            # ---------- phase D: out_proj partials (per finished q-tile) ----
            def d_tile(tt):
                for ob in range(8):
                    if True:
                        def op_item(ob=ob, tt=tt):
                            ts_ = slice(tt * 512, tt * 512 + 512)
                            pp = psA.tile([128, 512], F32, tag="proj",
                                          name=f"op_{tt}_{ob}")
                            nc.tensor.matmul(pp, wo_sb[:, ob, :], yT[:, ts_],
                                             start=True, stop=True)
                            ot = otp.tile([128, 512], BF16, tag="ot",
                                          name=f"ot_{tt}_{ob}")
                            if (ob + tt) % 2 == 0:
                                nc.vector.tensor_copy(ot, pp)
                            else:
                                nc.scalar.copy(ot, pp)
                            nc.sync.dma_start(
                                out=outT[ob * 128:(ob + 1) * 128, ts_], in_=ot)
                        op_item.act = (ob + tt) % 2 == 1
                        yield op_item

            # ---------- phase C: attention for batch b ----------
            def attention(b, fillB, fillD, pre=None):
                fillP = deque()   # own-batch projections, highest priority
                fillA = deque()   # deferred ACT-bound drains

                def popD(n):
                    # out_proj items; ACT-bound drains go to fillA instead so
                    # they never sit between exps in the ACT queue mid-q-tile
                    while n and fillD:
                        it = fillD.popleft()
                        if getattr(it, "act", False):
                            fillA.append(it)
                        else:
                            it()
                            n -= 1
                    return n

                def pop2(act_ok=False):
                    # two filler items per slot; own-batch projections first,
                    # then one next-batch projection + one out_proj item.
                    # B before D: freshly-added out_proj items depend on the
                    # yT normalize that only just ran on DVE.
                    n = 2
                    if act_ok and fillA:
                        fillA.popleft()()
                        n -= 1
                    while fillP and n:
                        fillP.popleft()()
                        n -= 1
                    if n and fillB:
                        fillB.popleft()()
                        n -= 1
                    # out_proj backlog: early batches bank their items so the
                    # last two (which run without next-batch projections in
                    # fillB) pop mature items whose yT was normalized long
                    # ago; popping fresh items stalls the PE on the DVE
                    # normalize
                    if len(fillD) > (16 if b < B - 1 else 0):
                        n = popD(n)

                sps = {}
                opsq = {}

                def emit_S(qt, kb):
                    # diagonal k-blocks only cover local q >= 128*jj
                    nkb = 4 * qt + 4
                    q0 = b * T + qt * 512
                    diag = kb >= nkb - 4
                    jj = max(0, kb - (nkb - 4))
                    lo = 128 * jj
                    ksl = slice(b * T + kb * 128, b * T + kb * 128 + 128)
                    sp = psS.tile([128, 2, 512], F32, tag="s",
                                  name=f"s_{b}_{qt}_{kb}")
                    nc.tensor.matmul(sp[:, 0, lo:512], KT[0:64, ksl],
                                     QT[0:64, q0 + lo:q0 + 512],
                                     start=True, stop=not diag,
                                     tile_position=(0, 0))
                    nc.tensor.matmul(sp[:, 1, lo:512], KT[64:128, ksl],
                                     QT[64:128, q0 + lo:q0 + 512],
                                     start=True, stop=not diag,
                                     tile_position=(64, 0))
                    if diag:
                        # causal mask folded into S: accumulate -256 on the
                        # strict upper triangle of the partial block (exp
                        # then yields ~3e-13 there, i.e. masked)
                        for j in range(2):
                            nc.tensor.matmul(
                                sp[:, j, lo:lo + 128], btri_sb, ident,
                                start=False, stop=True)
                    sps[(qt, kb)] = sp

                # flat (qt, kb) sequence: the S prefetch crosses q-tile
                # boundaries, so ACT's exp stream never drains at them
                flat = [(qt, kb) for qt in range(NQT)
                        for kb in range(4 * qt + 4)]
                emit_S(0, 0)
                for i, (qt, kb) in enumerate(flat):
                    nkb = 4 * qt + 4
                    q0 = b * T + qt * 512
                    if kb == 0:
                        if pre is not None and qt + 1 < NQT:
                            # the NEXT token-tile's own-batch projections are
                            # fully emitted before q-tile qt+1 consumes them
                            fillP.extend(pre[qt + 1])
                        opsq[qt] = [psO.tile([128, 512], F32, tag="o",
                                             name=f"o_{b}_{qt}_{j}")
                                    for j in range(2)]
                    ops = opsq[qt]
                    if i + 1 < len(flat):
                        emit_S(*flat[i + 1])
                    sp = sps.pop((qt, kb))
                    jj = max(0, kb - (nkb - 4))
                    lo = 128 * jj
                    pt = ptp.tile([128, 2, 512], BF16, tag="pt",
                                  name=f"pt_{b}_{qt}_{kb}")
                    nc.scalar.activation(pt[:, :, lo:512], sp[:, :, lo:512],
                                         AT.Exp, scale=0.125)
                    # fillers BETWEEN S(next) and PV(kb) on the PE queue:
                    # they hide exp(kb)'s latency + sem propagation so the
                    # PE never head-of-line blocks on the PV dependency.
                    # ACT-bound drains only during diagonal iterations,
                    # whose trimmed exps leave the ACT queue slack.
                    pop2(act_ok=kb >= nkb - 4)
                    gkb = b * NKB + kb
                    for j in range(2):
                        nc.tensor.matmul(
                            ops[j][:, lo:512], vtm[:, gkb, j, :],
                            pt[:, j, lo:512],
                            start=(kb == 0), stop=(kb == nkb - 1))
                    if kb == nkb - 1:
                        # normalize: O / denom -> yT (bf16), all on DVE:
                        # recip of the replicated denominator block + multiply
                        qsl = slice(q0, q0 + 512)
                        for j in range(2):
                            hsl = slice(64 * j, 64 * j + 64)
                            recip = small.tile([64, 512], F32, tag="rc",
                                               name=f"rc_{b}_{qt}_{j}")
                            nc.vector.reciprocal(recip, ops[j][64:128, :])
                            nc.vector.tensor_tensor(
                                out=yT[hsl, qsl], in0=ops[j][0:64, :],
                                in1=recip, op=OP.mult)
                        del opsq[qt]
                        # this q-tile's out_proj partials are now computable
                        fillD.extend(d_tile(NQT * b + qt))
                        pop2(act_ok=True)
                        pop2(act_ok=True)
                        # drain own-batch projections the kb loop didn't cover
                        while fillP:
                            fillP.popleft()()
                # deferred ACT drains must not leak out of this batch's
                # scope; popD first on the last batch (it refills fillA)
                if b == B - 1:
                    while fillD:
                        popD(len(fillD))
                        while fillA:
                            fillA.popleft()()
                while fillA:
                    fillA.popleft()()

            fillB = deque()
            fillD = deque()
            # batch 0: only token-tile 0's projections up front; tiles 1-3
            # interleave into the attention q-tile loop as priority fillers
            items0 = list(b_items(0))
            assert len(items0) == 12 * NQT
            chunks0 = [items0[12 * i:12 * i + 12] for i in range(NQT)]
            for i, it in enumerate(chunks0[0]):
                it()
                if rep == 0 and i in staged_dmas:
                    staged_dmas[i]()
            for b in range(B):
                if b + 1 < B:
                    fillB.extend(b_items(b + 1))
                attention(b, fillB, fillD, pre=chunks0 if b == 0 else None)
                # next batch's projections must fully precede its attention
                while fillB:
                    fillB.popleft()()
            while fillD:
                fillD.popleft()()

        for rep in range(repeat):
            emit_body(rep)

    nc.compile()
    return nc


def _get_nc():
    global _NC
    if _NC is None:
        _NC = _build_nc()
    return _NC


def _prep_in_maps(x, Wq, bq, Wk, bk, Wv, bv, Wo, bo):
    bf = ml_dtypes.bfloat16
    # x^T, bf16-rounded (matches reference's x.astype(bf16) exactly)
    xT = np.ascontiguousarray(
        np.asarray(x, np.float32).reshape(BT, C).astype(bf).T
    )

    # RoPE caches; rows [cos|sin|cos|sin] and [sin|cos|sin|cos]
    inv = (1.0 / 10000.0 ** (np.arange(0, DH, 2, dtype=np.float64) / DH))
    pos = np.arange(T, dtype=np.float64)
    fr = np.outer(pos, inv)                      # [T, 32]
    cosT = np.cos(fr).T.astype(np.float32)       # [32, T]
    sinT = np.sin(fr).T.astype(np.float32)
    csa = np.ascontiguousarray(
        np.concatenate([cosT, sinT, cosT, sinT], 0).astype(bf))
    csb = np.ascontiguousarray(
        np.concatenate([sinT, cosT, sinT, cosT], 0).astype(bf))

    # causal bias for the partial diagonal block: S += btri[n, m] at out[m, n],
    # -256 where q < k so exp(S/8 - 32) == 0 in bf16 terms
    ri = np.arange(128)[:, None]
    ci = np.arange(128)[None, :]
    btri = np.where(ri < ci, -256.0, 0.0).astype(bf)   # [128, 128]

    # RoPE combine matrices: rot = Ca^T t_a + Cb^T t_b
    ca = np.zeros((128, 128), np.float32)
    cb = np.zeros((128, 128), np.float32)
    for base in (0, 64):
        for m in range(32):
            ca[base + m, base + m] = 1.0          # E*cos
            ca[base + m + 32, base + m] = -1.0    # -O*sin
            cb[base + m, base + m + 32] = 1.0     # E*sin
            cb[base + m + 32, base + m + 32] = 1.0  # O*cos
    ca = ca.astype(bf)
    cb = cb.astype(bf)

    perm = np.concatenate([np.arange(0, DH, 2), np.arange(1, DH, 2)])
    Wq = np.asarray(Wq, np.float32)
    Wk = np.asarray(Wk, np.float32)
    Wv = np.asarray(Wv, np.float32)
    Wo = np.asarray(Wo, np.float32)
    bq = np.asarray(bq, np.float32)
    bk = np.asarray(bk, np.float32)
    bv = np.asarray(bv, np.float32)

    in_maps = []
    for c in range(NCORES):
        h0, h1 = 2 * c, 2 * c + 1
        cols = np.concatenate([DH * h0 + perm, DH * h1 + perm])
        in_maps.append({
            "xT": xT,
            "wq": np.ascontiguousarray(Wq[:, cols].astype(bf)),
            "wk": np.ascontiguousarray(Wk[:, cols].astype(bf)),
            "wv": np.ascontiguousarray(Wv[:, 128 * c:128 * c + 128].astype(bf)),
            "wo": np.ascontiguousarray(Wo[128 * c:128 * c + 128, :].astype(bf)),
            "bq": np.ascontiguousarray(bq[cols].reshape(128, 1)),
            "bk": np.ascontiguousarray(bk[cols].reshape(128, 1)),
            "bv": np.ascontiguousarray(
                bv[128 * c:128 * c + 128].reshape(128, 1)),
            "csa": csa, "csb": csb, "btri": btri, "ca": ca, "cb": cb,
        })
    return in_maps


def _gather(results, bo):
    acc = results[0]["outT"].astype(np.float32)
    for c in range(1, NCORES):
        acc = acc + results[c]["outT"].astype(np.float32)
    out = acc.T.reshape(B, T, C) + np.asarray(bo, np.float32)
    return np.ascontiguousarray(out.astype(np.float32))


def kernel(x, Wq, bq, Wk, bk, Wv, bv, Wo, bo):
    nc = _get_nc()
    in_maps = _prep_in_maps(x, Wq, bq, Wk, bk, Wv, bv, Wo, bo)
    res = run_bass_kernel_spmd(nc, in_maps, list(range(NCORES)))
    return _gather(res.results, bo)



# revision 70
# speedup vs baseline: 1.3911x; 1.3911x over previous
"""Causal self-attention (B=4, T=2048, C=1024, H=16) on 8 TRN2 NeuronCores.

Sharding: tensor-parallel over heads. Core c owns heads (2c, 2c+1) for all
batches: QKV projections are column-sharded, attention is embarrassingly
parallel over (batch, head), out_proj is contraction-sharded and the host
sums the 8 partial outputs (the unshard step for a contraction shard).

Single-core schedule: a software pipeline over batches.  For each batch b:
projections+RoPE for b's 2048 tokens, then attention (both heads), with the
NEXT batch's projection work and the PREVIOUS batch's out_proj tiles
interleaved into the attention kb-loop as "filler" so the PE never idles
while the ACT engine runs the softmax exp stream.

Key device-level tricks:
  - S^T tiles for the two heads are computed by a row-tiled PAIR of matmuls
    (tile_position (0,0) and (64,0)): each head contracts over its own 64
    partitions, writing the two halves of one [128, 2, 512] PSUM tile.
  - Causal handling: S/exp/PV are column-trimmed to the valid q-suffix on
    diagonal k-blocks, and the within-block staircase is folded INTO the S
    accumulation as a -256 strict-upper-triangular bias matmul (btri^T @ I),
    so exp(S/8 - 32) == 0 there and no post-exp mask op exists at all.
  - exp runs on ACT over the trimmed [128, 2, <=512] view with scale=1/8;
    max-subtraction is skipped (|S|/8 <= ~3.2 for this operator).
  - V is transposed token-major 128x128 (both heads at once) on the PE; the
    vtm layout [128, gkb, 2, 128] holds each head's 64 dims + a 64-wide
    ones block, so the PV matmul emits O^T rows AND the softmax denominator
    REPLICATED on partitions 64:128 - normalize is then just a DVE
    reciprocal + multiply, with no cross-partition broadcast.
  - out_proj partials are written bf16 (halves the output DMA); the host
    sums the 8 partials in fp32, adds bo, and transposes back.
  - Scheduling: a flat (q-tile, k-block) software pipeline with the S pair
    emitted one step ahead, filler items (next-batch projections, banked
    out_proj tiles) popped between S and PV to hide exp latency, ACT-bound
    drains deferred to diagonal iterations, and DMA constants staged
    just-in-time behind the first x tile.
"""

import numpy as np
import ml_dtypes

import concourse.bass as bass
import concourse.mybir as mybir
import concourse.tile as tile
from concourse import bacc
from concourse.bass_utils import run_bass_kernel_spmd
from concourse.masks import make_identity

BF16 = mybir.dt.bfloat16
F32 = mybir.dt.float32
AT = mybir.ActivationFunctionType
OP = mybir.AluOpType

B, T, C, H = 4, 2048, 1024, 16
DH = 64
BT = B * T            # 8192
NCORES = 8
NKB = T // 128        # 16 k-blocks per batch
NQT = T // 512        # 4 q-tiles per batch

_NC = None            # cached compiled Bass module


def _build_nc(repeat=1, phases="BCD"):
    nc = bacc.Bacc("TRN2", target_bir_lowering=False, debug=False)

    xT = nc.declare_dram_parameter("xT", [C, BT], BF16, isOutput=False)
    # weights arrive pre-rearranged [partition, block, 128] so their DMAs
    # are one contiguous 2KB run per partition instead of a strided gather
    wq = nc.declare_dram_parameter("wq", [128, 8, 128], BF16, isOutput=False)
    wk = nc.declare_dram_parameter("wk", [128, 8, 128], BF16, isOutput=False)
    wv = nc.declare_dram_parameter("wv", [128, 8, 128], BF16, isOutput=False)
    wo = nc.declare_dram_parameter("wo", [128, 8, 128], BF16, isOutput=False)
    bq = nc.declare_dram_parameter("bq", [128, 1], F32, isOutput=False)
    bk = nc.declare_dram_parameter("bk", [128, 1], F32, isOutput=False)
    bv = nc.declare_dram_parameter("bv", [128, 1], F32, isOutput=False)
    csa = nc.declare_dram_parameter("csa", [128, T], BF16, isOutput=False)
    csb = nc.declare_dram_parameter("csb", [128, T], BF16, isOutput=False)
    btri = nc.declare_dram_parameter("btri", [128, 128], BF16, isOutput=False)
    ca = nc.declare_dram_parameter("ca", [128, 128], BF16, isOutput=False)
    cb = nc.declare_dram_parameter("cb", [128, 128], BF16, isOutput=False)
    outT = nc.declare_dram_parameter("outT", [C, BT], BF16, isOutput=True)

    from contextlib import ExitStack
    from collections import deque
    with tile.TileContext(nc) as tc, ExitStack() as ctx:
        const = ctx.enter_context(tc.tile_pool(name="const", bufs=1))
        xpool = ctx.enter_context(tc.tile_pool(name="xpool", bufs=4))
        ptp = ctx.enter_context(tc.tile_pool(name="ptp", bufs=8))
        rtmp = ctx.enter_context(tc.tile_pool(name="rtmp", bufs=4))
        small = ctx.enter_context(tc.tile_pool(name="small", bufs=3))
        otp = ctx.enter_context(tc.tile_pool(name="otp", bufs=4))
        psA = ctx.enter_context(tc.tile_pool(name="psA", bufs=2, space="PSUM"))
        psS = ctx.enter_context(tc.tile_pool(name="psS", bufs=2, space="PSUM"))
        psO = ctx.enter_context(tc.tile_pool(name="psO", bufs=2, space="PSUM"))

        # ---- constants ----
        # Only what batch 0 / token-tile 0's first matmuls need is loaded up
        # front; everything else is staged between the first token-tile's
        # filler items (the HWDGE serves DMAs roughly in issue order, so a
        # big table in front of the first x tile delays the whole pipeline).
        wq_sb = const.tile([128, 8, 128], BF16, tag="wq")
        nc.sync.dma_start(out=wq_sb, in_=wq[:, :, :])
        wk_sb = const.tile([128, 8, 128], BF16, tag="wk")
        wv_sb = const.tile([128, 8, 128], BF16, tag="wv")
        wo_sb = const.tile([128, 8, 128], BF16, tag="wo")
        csa_sb = const.tile([128, T], BF16, tag="csa")
        csb_sb = const.tile([128, T], BF16, tag="csb")
        btri_sb = const.tile([128, 128], BF16, tag="btri")
        ca_sb = const.tile([128, 128], BF16, tag="ca")
        cb_sb = const.tile([128, 128], BF16, tag="cb")
        bq_sb = const.tile([128, 1], F32, tag="bq")
        nc.sync.dma_start(out=bq_sb, in_=bq[:, :])
        bk_sb = const.tile([128, 1], F32, tag="bk")
        bv_sb = const.tile([128, 1], F32, tag="bv")
        nc.sync.dma_start(out=btri_sb, in_=btri[:, :])

        staged_dmas = {
            0: lambda: (
                nc.sync.dma_start(out=csa_sb[:, 0:512], in_=csa[:, 0:512]),
                nc.sync.dma_start(out=csb_sb[:, 0:512], in_=csb[:, 0:512]),
                nc.sync.dma_start(out=ca_sb, in_=ca[:, :]),
                nc.sync.dma_start(out=cb_sb, in_=cb[:, :]),
            ),
            1: lambda: (
                nc.sync.dma_start(out=wk_sb, in_=wk[:, :, :]),
                nc.sync.dma_start(out=bk_sb, in_=bk[:, :]),
            ),
            3: lambda: (
                nc.sync.dma_start(out=wv_sb, in_=wv[:, :, :]),
                nc.sync.dma_start(out=bv_sb, in_=bv[:, :]),
            ),
            5: lambda: (
                nc.sync.dma_start(out=csa_sb[:, 512:T], in_=csa[:, 512:T]),
                nc.sync.dma_start(out=csb_sb[:, 512:T], in_=csb[:, 512:T]),
            ),
            6: lambda: (
                nc.sync.dma_start(out=wo_sb, in_=wo[:, :, :]),
            ),
        }

        ident = const.tile([128, 128], BF16, tag="id")
        make_identity(nc, ident)

        QT = const.tile([128, BT], BF16, tag="QT")
        KT = const.tile([128, BT], BF16, tag="KT")
        VT = const.tile([128, BT], BF16, tag="VT")
        yT = const.tile([128, BT], BF16, tag="yT")
        # token-major V: [k-part, gkb, head, 64 dims + 64 ones]. The ones
        # block makes the PV matmul emit the softmax denominator replicated
        # on partitions 64:128, so normalize needs no partition_broadcast.
        vtm = const.tile([128, B * NKB, 2, 128], BF16, tag="vtm")
        nc.vector.memset(vtm[:, :, :, 64:128], 1.0)

        def emit_body(rep=0):
            # ---------- phase B: projections + RoPE + V transpose ----------
            def b_items(b):
                """Generator of filler closures for batch b's projections.

                All tile() allocations happen inside the closures (at pop
                time) so pool-slot WAR deps follow emission order; tiles
                shared between closures of one token-tile pass via `cell`.
                """
                for tt in range(NQT * b, NQT * b + NQT):
                    ts_ = slice(tt * 512, tt * 512 + 512)
                    pos = slice((tt % 4) * 512, (tt % 4) * 512 + 512)
                    cell = {}

                    def dma_x(cell=cell, ts_=ts_, tt=tt):
                        xt = xpool.tile([128, 8, 512], BF16, tag="xt",
                                        name=f"xt_{tt}")
                        cell["xt"] = xt
                        # Alternate the two HWDGE trigger queues so x loads
                        # overlap each other and never all serialize behind
                        # the outT stores or the big constant DMAs. Two-half
                        # split: mm4(0) only needs k-blocks 0:4.
                        eng = nc.scalar if tt % 2 == 0 else nc.sync
                        src = xT.rearrange("(kb p) m -> p kb m", p=128)
                        eng.dma_start(out=xt[:, 0:4, :], in_=src[:, 0:4, ts_])
                        eng.dma_start(out=xt[:, 4:8, :], in_=src[:, 4:8, ts_])
                    yield dma_x

                    def make_proj(w_sb, b_sb, dstT, cell=cell, tt=tt, pos=pos,
                                  ts_=ts_):
                        pk = f"pp_{dstT.name}"

                        def mm4(lo):
                            if lo == 0:
                                cell[pk] = psA.tile([128, 512], F32, tag="proj",
                                                    name=f"{pk}_{tt}")
                            pp = cell[pk]
                            xt = cell["xt"]
                            for kb in range(lo, lo + 4):
                                nc.tensor.matmul(pp, w_sb[:, kb, :], xt[:, kb, :],
                                                 start=(kb == 0), stop=(kb == 7))

                        def r_stst():
                            # DVE half of RoPE, issued one filler slot before
                            # the PE combine so the stst latency is hidden
                            pp = cell[pk]
                            ta = rtmp.tile([128, 512], BF16, tag="ta",
                                           name=f"ta_{tt}_{dstT.name}")
                            tb = rtmp.tile([128, 512], BF16, tag="tb",
                                           name=f"tb_{tt}_{dstT.name}")
                            cell[pk + "ab"] = (ta, tb)
                            nc.vector.scalar_tensor_tensor(
                                out=ta, in0=pp, scalar=b_sb[:, 0:1],
                                in1=csa_sb[:, pos], op0=OP.add, op1=OP.mult)
                            nc.vector.scalar_tensor_tensor(
                                out=tb, in0=pp, scalar=b_sb[:, 0:1],
                                in1=csb_sb[:, pos], op0=OP.add, op1=OP.mult)

                        def mm4b_stst():
                            mm4(4)
                            r_stst()

                        def r_epi():
                            ta, tb = cell[pk + "ab"]
                            rp = psA.tile([128, 512], F32, tag="proj",
                                          name=f"rp_{tt}_{dstT.name}")
                            nc.tensor.matmul(rp, ca_sb, ta, start=True,
                                             stop=False)
                            nc.tensor.matmul(rp, cb_sb, tb, start=False,
                                             stop=True)
                            nc.vector.tensor_copy(dstT[:, ts_], rp)

                        def v_epi():
                            nc.vector.tensor_scalar_add(VT[:, ts_], cell[pk],
                                                        b_sb[:, 0:1])

                        return mm4, mm4b_stst, r_epi, v_epi

                    q_mm4, q_mm4b, q_epi, _ = make_proj(wq_sb, bq_sb, QT)
                    k_mm4, k_mm4b, k_epi, _ = make_proj(wk_sb, bk_sb, KT)
                    v_mm4, _, _, v_epi = make_proj(wv_sb, bv_sb, VT)

                    def v_tr_fn(half, tt=tt):
                        def v_tr():
                            tr = psA.tile([128, 512], F32, tag="proj",
                                          name=f"tr_{tt}_{half}")
                            trb = tr.bitcast(BF16)  # [128, 1024] bf16 view
                            for s in range(2):
                                sub = half * 2 + s
                                gkb = tt * 4 + sub
                                col = slice(tt * 512 + sub * 128,
                                            tt * 512 + sub * 128 + 128)
                                nc.tensor.transpose(
                                    trb[:, 128 * s:128 * s + 128],
                                    VT[:, col], ident)
                                nc.vector.tensor_copy(
                                    vtm[:, gkb, :, 0:64],
                                    trb[:, 128 * s:128 * s + 128]
                                    .rearrange("p (j d) -> p j d", j=2))
                        return v_tr

                    # order interleaves the three projections so every DVE
                    # producer (stst, v_epi) has a PE item between it and its
                    # PE consumer (rp combine / transpose)
                    yield (lambda f=q_mm4: f(0))
                    yield q_mm4b
                    yield (lambda f=k_mm4: f(0))
                    yield q_epi
                    yield k_mm4b
                    yield (lambda f=v_mm4: f(0))
                    yield k_epi
                    yield (lambda f=v_mm4: f(4))
                    yield v_epi
                    yield v_tr_fn(0)
                    yield v_tr_fn(1)

            # ---------- phase D: out_proj partials (per finished q-tile) ----
            def d_tile(tt):
                for ob in range(8):
                    if True:
                        def op_item(ob=ob, tt=tt):
                            ts_ = slice(tt * 512, tt * 512 + 512)
                            pp = psA.tile([128, 512], F32, tag="proj",
                                          name=f"op_{tt}_{ob}")
                            nc.tensor.matmul(pp, wo_sb[:, ob, :], yT[:, ts_],
                                             start=True, stop=True)
                            ot = otp.tile([128, 512], BF16, tag="ot",
                                          name=f"ot_{tt}_{ob}")
                            if (ob + tt) % 2 == 0:
                                nc.vector.tensor_copy(ot, pp)
                            else:
                                nc.scalar.copy(ot, pp)
                            nc.sync.dma_start(
                                out=outT[ob * 128:(ob + 1) * 128, ts_], in_=ot)
                        op_item.act = (ob + tt) % 2 == 1
                        yield op_item

            # ---------- phase C: attention for batch b ----------
            def attention(b, fillB, fillD, pre=None):
                fillP = deque()   # own-batch projections, highest priority
                fillA = deque()   # deferred ACT-bound drains

                def popD(n):
                    # out_proj items; ACT-bound drains go to fillA instead so
                    # they never sit between exps in the ACT queue mid-q-tile
                    while n and fillD:
                        it = fillD.popleft()
                        if getattr(it, "act", False):
                            fillA.append(it)
                        else:
                            it()
                            n -= 1
                    return n

                def pop2(act_ok=False):
                    # two filler items per slot; own-batch projections first,
                    # then one next-batch projection + one out_proj item.
                    # B before D: freshly-added out_proj items depend on the
                    # yT normalize that only just ran on DVE.
                    n = 2
                    if act_ok and fillA:
                        fillA.popleft()()
                        n -= 1
                    while fillP and n:
                        fillP.popleft()()
                        n -= 1
                    if n and fillB:
                        fillB.popleft()()
                        n -= 1
                    # out_proj backlog: early batches bank their items so the
                    # last two (which run without next-batch projections in
                    # fillB) pop mature items whose yT was normalized long
                    # ago; popping fresh items stalls the PE on the DVE
                    # normalize
                    if len(fillD) > (16 if b < B - 1 else 0):
                        n = popD(n)

                sps = {}
                opsq = {}

                def emit_S(qt, kb):
                    # diagonal k-blocks only cover local q >= 128*jj
                    nkb = 4 * qt + 4
                    q0 = b * T + qt * 512
                    diag = kb >= nkb - 4
                    jj = max(0, kb - (nkb - 4))
                    lo = 128 * jj
                    ksl = slice(b * T + kb * 128, b * T + kb * 128 + 128)
                    sp = psS.tile([128, 2, 512], F32, tag="s",
                                  name=f"s_{b}_{qt}_{kb}")
                    nc.tensor.matmul(sp[:, 0, lo:512], KT[0:64, ksl],
                                     QT[0:64, q0 + lo:q0 + 512],
                                     start=True, stop=not diag,
                                     tile_position=(0, 0))
                    nc.tensor.matmul(sp[:, 1, lo:512], KT[64:128, ksl],
                                     QT[64:128, q0 + lo:q0 + 512],
                                     start=True, stop=not diag,
                                     tile_position=(64, 0))
                    if diag:
                        # causal mask folded into S: accumulate -256 on the
                        # strict upper triangle of the partial block (exp
                        # then yields ~3e-13 there, i.e. masked)
                        for j in range(2):
                            nc.tensor.matmul(
                                sp[:, j, lo:lo + 128], btri_sb, ident,
                                start=False, stop=True)
                    sps[(qt, kb)] = sp

                # flat (qt, kb) sequence: the S prefetch crosses q-tile
                # boundaries, so ACT's exp stream never drains at them
                flat = [(qt, kb) for qt in range(NQT)
                        for kb in range(4 * qt + 4)]
                emit_S(0, 0)
                for i, (qt, kb) in enumerate(flat):
                    nkb = 4 * qt + 4
                    q0 = b * T + qt * 512
                    if kb == 0:
                        if pre is not None and qt + 1 < NQT:
                            # the NEXT token-tile's own-batch projections are
                            # fully emitted before q-tile qt+1 consumes them
                            fillP.extend(pre[qt + 1])
                        opsq[qt] = [psO.tile([128, 512], F32, tag="o",
                                             name=f"o_{b}_{qt}_{j}")
                                    for j in range(2)]
                    ops = opsq[qt]
                    if i + 1 < len(flat):
                        emit_S(*flat[i + 1])
                    sp = sps.pop((qt, kb))
                    jj = max(0, kb - (nkb - 4))
                    lo = 128 * jj
                    pt = ptp.tile([128, 2, 512], BF16, tag="pt",
                                  name=f"pt_{b}_{qt}_{kb}")
                    nc.scalar.activation(pt[:, :, lo:512], sp[:, :, lo:512],
                                         AT.Exp, scale=0.125)
                    # fillers BETWEEN S(next) and PV(kb) on the PE queue:
                    # they hide exp(kb)'s latency + sem propagation so the
                    # PE never head-of-line blocks on the PV dependency.
                    # ACT-bound drains only during diagonal iterations,
                    # whose trimmed exps leave the ACT queue slack.
                    pop2(act_ok=kb >= nkb - 4)
                    gkb = b * NKB + kb
                    for j in range(2):
                        nc.tensor.matmul(
                            ops[j][:, lo:512], vtm[:, gkb, j, :],
                            pt[:, j, lo:512],
                            start=(kb == 0), stop=(kb == nkb - 1))
                    if kb == nkb - 1:
                        qsl = slice(q0, q0 + 512)
                        if b == B - 1 and qt == NQT - 1:
                            # final q-tile: nothing left to overlap with, so
                            # pipeline the tail in token-halves (normalize
                            # half A, then its out_proj while B normalizes)
                            for half in range(2):
                                c0 = 256 * half
                                csl = slice(q0 + c0, q0 + c0 + 256)
                                for j in range(2):
                                    hsl = slice(64 * j, 64 * j + 64)
                                    rc = small.tile(
                                        [64, 256], F32, tag="rch",
                                        name=f"rch_{j}_{half}")
                                    nc.vector.reciprocal(
                                        rc, ops[j][64:128, c0:c0 + 256])
                                    nc.vector.tensor_tensor(
                                        out=yT[hsl, csl],
                                        in0=ops[j][0:64, c0:c0 + 256],
                                        in1=rc, op=OP.mult)
                                for ob in range(8):
                                    pp = psA.tile([128, 512], F32, tag="proj",
                                                  name=f"oph_{ob}_{half}")
                                    nc.tensor.matmul(pp[:, 0:256],
                                                     wo_sb[:, ob, :],
                                                     yT[:, csl],
                                                     start=True, stop=True)
                                    ot = otp.tile([128, 512], BF16, tag="ot",
                                                  name=f"oth_{ob}_{half}")
                                    if ob % 2 == 0:
                                        nc.vector.tensor_copy(ot[:, 0:256],
                                                              pp[:, 0:256])
                                    else:
                                        nc.scalar.copy(ot[:, 0:256],
                                                       pp[:, 0:256])
                                    nc.sync.dma_start(
                                        out=outT[ob * 128:(ob + 1) * 128,
                                                 csl],
                                        in_=ot[:, 0:256])
                            del opsq[qt]
                            while fillP:
                                fillP.popleft()()
                            continue
                        # normalize: O / denom -> yT (bf16), all on DVE:
                        # recip of the replicated denominator block + multiply
                        for j in range(2):
                            hsl = slice(64 * j, 64 * j + 64)
                            recip = small.tile([64, 512], F32, tag="rc",
                                               name=f"rc_{b}_{qt}_{j}")
                            nc.vector.reciprocal(recip, ops[j][64:128, :])
                            nc.vector.tensor_tensor(
                                out=yT[hsl, qsl], in0=ops[j][0:64, :],
                                in1=recip, op=OP.mult)
                        del opsq[qt]
                        # this q-tile's out_proj partials are now computable
                        fillD.extend(d_tile(NQT * b + qt))
                        pop2(act_ok=True)
                        pop2(act_ok=True)
                        # drain own-batch projections the kb loop didn't cover
                        while fillP:
                            fillP.popleft()()
                # deferred ACT drains must not leak out of this batch's
                # scope; popD first on the last batch (it refills fillA)
                if b == B - 1:
                    while fillD:
                        popD(len(fillD))
                        while fillA:
                            fillA.popleft()()
                while fillA:
                    fillA.popleft()()

            fillB = deque()
            fillD = deque()
            # batch 0: only token-tile 0's projections up front; tiles 1-3
            # interleave into the attention q-tile loop as priority fillers
            items0 = list(b_items(0))
            assert len(items0) == 12 * NQT
            chunks0 = [items0[12 * i:12 * i + 12] for i in range(NQT)]
            for i, it in enumerate(chunks0[0]):
                it()
                if rep == 0 and i in staged_dmas:
                    staged_dmas[i]()
            for b in range(B):
                if b + 1 < B:
                    fillB.extend(b_items(b + 1))
                attention(b, fillB, fillD, pre=chunks0 if b == 0 else None)
                # next batch's projections must fully precede its attention
                while fillB:
                    fillB.popleft()()
            while fillD:
                fillD.popleft()()

        for rep in range(repeat):
            emit_body(rep)

    nc.compile()
    return nc


def _get_nc():
    global _NC
    if _NC is None:
        _NC = _build_nc()
    return _NC


def _prep_in_maps(x, Wq, bq, Wk, bk, Wv, bv, Wo, bo):
    bf = ml_dtypes.bfloat16
    # x^T, bf16-rounded (matches reference's x.astype(bf16) exactly)
    xT = np.ascontiguousarray(
        np.asarray(x, np.float32).reshape(BT, C).astype(bf).T
    )

    # RoPE caches; rows [cos|sin|cos|sin] and [sin|cos|sin|cos]
    inv = (1.0 / 10000.0 ** (np.arange(0, DH, 2, dtype=np.float64) / DH))
    pos = np.arange(T, dtype=np.float64)
    fr = np.outer(pos, inv)                      # [T, 32]
    cosT = np.cos(fr).T.astype(np.float32)       # [32, T]
    sinT = np.sin(fr).T.astype(np.float32)
    csa = np.ascontiguousarray(
        np.concatenate([cosT, sinT, cosT, sinT], 0).astype(bf))
    csb = np.ascontiguousarray(
        np.concatenate([sinT, cosT, sinT, cosT], 0).astype(bf))

    # causal bias for the partial diagonal block: S += btri[n, m] at out[m, n],
    # -256 where q < k so exp(S/8 - 32) == 0 in bf16 terms
    ri = np.arange(128)[:, None]
    ci = np.arange(128)[None, :]
    btri = np.where(ri < ci, -256.0, 0.0).astype(bf)   # [128, 128]

    # RoPE combine matrices: rot = Ca^T t_a + Cb^T t_b
    ca = np.zeros((128, 128), np.float32)
    cb = np.zeros((128, 128), np.float32)
    for base in (0, 64):
        for m in range(32):
            ca[base + m, base + m] = 1.0          # E*cos
            ca[base + m + 32, base + m] = -1.0    # -O*sin
            cb[base + m, base + m + 32] = 1.0     # E*sin
            cb[base + m + 32, base + m + 32] = 1.0  # O*cos
    ca = ca.astype(bf)
    cb = cb.astype(bf)

    perm = np.concatenate([np.arange(0, DH, 2), np.arange(1, DH, 2)])
    Wq = np.asarray(Wq, np.float32)
    Wk = np.asarray(Wk, np.float32)
    Wv = np.asarray(Wv, np.float32)
    Wo = np.asarray(Wo, np.float32)
    bq = np.asarray(bq, np.float32)
    bk = np.asarray(bk, np.float32)
    bv = np.asarray(bv, np.float32)

    in_maps = []
    for c in range(NCORES):
        h0, h1 = 2 * c, 2 * c + 1
        cols = np.concatenate([DH * h0 + perm, DH * h1 + perm])
        in_maps.append({
            "xT": xT,
            # [C, 128] -> [p, kb, 128]: row kb*128+p lands at [p, kb, :]
            "wq": np.ascontiguousarray(
                Wq[:, cols].astype(bf).reshape(8, 128, 128).transpose(1, 0, 2)),
            "wk": np.ascontiguousarray(
                Wk[:, cols].astype(bf).reshape(8, 128, 128).transpose(1, 0, 2)),
            "wv": np.ascontiguousarray(
                Wv[:, 128 * c:128 * c + 128].astype(bf)
                .reshape(8, 128, 128).transpose(1, 0, 2)),
            # [128, C] -> [p, ob, 128]: out-block ob's columns at [p, ob, :]
            "wo": np.ascontiguousarray(
                Wo[128 * c:128 * c + 128, :].astype(bf).reshape(128, 8, 128)),
            "bq": np.ascontiguousarray(bq[cols].reshape(128, 1)),
            "bk": np.ascontiguousarray(bk[cols].reshape(128, 1)),
            "bv": np.ascontiguousarray(
                bv[128 * c:128 * c + 128].reshape(128, 1)),
            "csa": csa, "csb": csb, "btri": btri, "ca": ca, "cb": cb,
        })
    return in_maps


def _gather(results, bo):
    acc = results[0]["outT"].astype(np.float32)
    for c in range(1, NCORES):
        acc = acc + results[c]["outT"].astype(np.float32)
    out = acc.T.reshape(B, T, C) + np.asarray(bo, np.float32)
    return np.ascontiguousarray(out.astype(np.float32))


def kernel(x, Wq, bq, Wk, bk, Wv, bv, Wo, bo):
    nc = _get_nc()
    in_maps = _prep_in_maps(x, Wq, bq, Wk, bk, Wv, bv, Wo, bo)
    res = run_bass_kernel_spmd(nc, in_maps, list(range(NCORES)))
    return _gather(res.results, bo)

